# revision 1
# baseline (speedup 1.0000x reference)
"""KNN overlap loss on 8 Trainium2 NeuronCores.

loss = 1 - |top15(input) ∩ top15(target)| / (N*k), per-row index-set overlap.

Strategy (row-sharded across 8 cores, 1250 rows/core, padded to 1280):
  Per 128-row block, per matrix m ∈ {input, target}:
    e_m[q, j] = x_q · x_j - 0.5||x_j||^2   (row-constant term dropped: does
    not change per-row top-k).  Computed as one K=128 matmul + one K=1
    matmul accumulating -0.5*sq[j] into the same PSUM tile (20 tiles x 500).
    Top-15-largest e == top-15-smallest distance.
  Selection without indices: per 500-wide segment take top-8 (DVE max8)
  -> 160 candidates/row.  c15, c16 = 15th/16th largest candidate
  (max8 + match_replace + max8).  Threshold t' = (c15+c16)/2.  Then
    overlap_row = sum_j [e_in >= t'_in] * sign(e_tgt - t'_tgt)  = 2*ov - 15.
  Exactness guard: z = max over segments of the segment's 8th-largest.
  If z >= t' (or c15 == c16) the candidate set may have missed a top-15
  member -> row flagged, host recomputes that row exactly (rare: P ~ 1e-6).
"""

import sys

sys.path.insert(0, "/opt/trn_rl_repo")

import numpy as np

N = 10000
D = 128
KNN = 15
NCORES = 8
RPC = N // NCORES          # rows per core = 1250
RPAD = 1280                # padded to 10 blocks of 128
NBLK = RPAD // 128         # 10
TW = 500                   # tile width
NT = N // TW               # 20 tiles
USE_F32R = True

_CACHE = {}


def _build():
    import concourse.bacc as bacc
    import concourse.mybir as mybir
    import concourse.tile as tile

    f32 = mybir.dt.float32
    fin = mybir.dt.float32r if USE_F32R else f32

    nc = bacc.Bacc(None, target_bir_lowering=False)

    xt_in = nc.dram_tensor("xt_in", [D, N], fin, kind="ExternalInput")
    xt_tg = nc.dram_tensor("xt_tg", [D, N], fin, kind="ExternalInput")
    q_in = nc.dram_tensor("q_in", [D, RPAD], fin, kind="ExternalInput")
    q_tg = nc.dram_tensor("q_tg", [D, RPAD], fin, kind="ExternalInput")
    ms_in = nc.dram_tensor("ms_in", [1, N], fin, kind="ExternalInput")
    ms_tg = nc.dram_tensor("ms_tg", [1, N], fin, kind="ExternalInput")
    ones = nc.dram_tensor("ones", [1, 128], fin, kind="ExternalInput")
    out_d = nc.dram_tensor("out", [RPAD, 8], f32, kind="ExternalOutput")

    with tile.TileContext(nc) as tc:
        with (
            tc.tile_pool(name="big", bufs=1) as big,
            tc.tile_pool(name="sm", bufs=2) as sm,
            tc.tile_pool(name="ps", bufs=3, space="PSUM") as ps,
        ):
            xt_in_t = big.tile([D, N], fin)
            xt_tg_t = big.tile([D, N], fin)
            e_in_t = big.tile([128, N], f32)
            e_tg_t = big.tile([128, N], f32)
            q_in_t = big.tile([D, RPAD], fin)
            q_tg_t = big.tile([D, RPAD], fin)
            ones_t = big.tile([1, 128], fin)
            nc.sync.dma_start(xt_in_t[:], xt_in[:])
            nc.sync.dma_start(xt_tg_t[:], xt_tg[:])
            nc.sync.dma_start(q_in_t[:], q_in[:])
            nc.sync.dma_start(q_tg_t[:], q_tg[:])
            nc.sync.dma_start(ones_t[:], ones[:])

            for b in range(NBLK):
                rs = slice(b * 128, (b + 1) * 128)
                # per-matrix phase A: matmul tiles -> PSUM -> SBUF + max8 cands
                stats = {}
                for (qt, xtt, msd, et, tagp) in (
                    (q_in_t, xt_in_t, ms_in, e_in_t, "pin"),
                    (q_tg_t, xt_tg_t, ms_tg, e_tg_t, "ptg"),
                ):
                    cands = sm.tile([128, NT * 8], f32, tag="cands" + tagp)
                    for t in range(NT):
                        cs = slice(t * TW, (t + 1) * TW)
                        pt = ps.tile([128, TW], f32, tag=tagp)
                        mstage = sm.tile([1, TW], fin, tag="ms" + tagp)
                        nc.sync.dma_start(mstage[:], msd[0:1, cs])
                        nc.tensor.matmul(
                            pt[:], qt[:, rs], xtt[:, cs], start=True, stop=False
                        )
                        nc.tensor.matmul(
                            pt[:], ones_t[:], mstage[:], start=False, stop=True
                        )
                        nc.scalar.copy(et[:, cs], pt[:])
                        nc.vector.max(cands[:, t * 8 : (t + 1) * 8], et[:, cs])
                    # threshold from candidates
                    m1 = sm.tile([128, 8], f32, tag="m1" + tagp)
                    mr = sm.tile([128, NT * 8], f32, tag="mr" + tagp)
                    m2 = sm.tile([128, 8], f32, tag="m2" + tagp)
                    zt = sm.tile([128, 8], f32, tag="zt" + tagp)
                    thr = sm.tile([128, 1], f32, tag="thr" + tagp)
                    nthr = sm.tile([128, 1], f32, tag="nthr" + tagp)
                    pre = sm.tile([128, 1], f32, tag="pre" + tagp)
                    nc.vector.max(m1[:], cands[:])
                    nc.vector.match_replace(mr[:], m1[:], cands[:], -1e38)
                    nc.vector.max(m2[:], mr[:])
                    c3 = cands[:].rearrange("p (s e) -> p s e", e=8)
                    nc.vector.max(zt[:], c3[:, :, 7:8])
                    nc.vector.tensor_tensor(
                        pre[:], m2[:, 6:7], m2[:, 7:8], mybir.AluOpType.add
                    )
                    nc.vector.tensor_scalar_mul(thr[:], pre[:], 0.5)
                    nc.vector.tensor_scalar_mul(nthr[:], pre[:], -0.5)
                    stats[tagp] = (thr, nthr, m2, zt)

                thrA, _, m2A, ztA = stats["pin"]
                thrB, nthrB, m2B, ztB = stats["ptg"]

                # phase B: acc_row = sum_j (e_in >= t'A) * sign(e_tg - t'B)
                slots = sm.tile([128, NT], f32, tag="slots")
                for t in range(NT):
                    cs = slice(t * TW, (t + 1) * TW)
                    sg = sm.tile([128, TW], f32, tag="sg")
                    jk = sm.tile([128, TW], f32, tag="jk")
                    nc.scalar.activation(
                        sg[:],
                        e_tg_t[:, cs],
                        mybir.ActivationFunctionType.Sign,
                        bias=nthrB[:],
                        scale=1.0,
                    )
                    nc.vector.scalar_tensor_tensor(
                        jk[:],
                        e_in_t[:, cs],
                        thrA[:],
                        sg[:],
                        mybir.AluOpType.is_ge,
                        mybir.AluOpType.mult,
                        accum_out=slots[:, t : t + 1],
                    )
                ob = sm.tile([128, 8], f32, tag="ob")
                nc.vector.reduce_sum(
                    ob[:, 0:1], slots[:], axis=mybir.AxisListType.X
                )
                nc.vector.tensor_copy(ob[:, 1:2], m2A[:, 6:7])
                nc.vector.tensor_copy(ob[:, 2:3], m2A[:, 7:8])
                nc.vector.tensor_copy(ob[:, 3:4], ztA[:, 0:1])
                nc.vector.tensor_copy(ob[:, 4:5], m2B[:, 6:7])
                nc.vector.tensor_copy(ob[:, 5:6], m2B[:, 7:8])
                nc.vector.tensor_copy(ob[:, 6:7], ztB[:, 0:1])
                nc.vector.tensor_copy(ob[:, 7:8], ztB[:, 0:1])
                nc.sync.dma_start(out_d[rs, :], ob[:])

    nc.finalize()
    return nc


def _host_row_overlap(x_in, x_tg, sq_in, sq_tg, r, k):
    d_in = sq_in[r] + sq_in - 2.0 * (x_in @ x_in[r])
    d_tg = sq_tg[r] + sq_tg - 2.0 * (x_tg @ x_tg[r])
    a = np.argsort(d_in, kind="stable")[:k]
    bb = np.argsort(d_tg, kind="stable")[:k]
    return len(set(a.tolist()) & set(bb.tolist()))


def kernel(input, target, k):
    from concourse.bass_utils import run_bass_kernel_spmd

    x_in = np.asarray(input, np.float32)
    x_tg = np.asarray(target, np.float32)
    k = int(k)
    sq_in = np.sum(x_in * x_in, axis=1)
    sq_tg = np.sum(x_tg * x_tg, axis=1)

    if k != KNN or x_in.shape != (N, D):
        total = sum(
            _host_row_overlap(x_in, x_tg, sq_in, sq_tg, r, k)
            for r in range(x_in.shape[0])
        )
        return np.float32(1.0 - total / np.float32(x_in.shape[0] * k))

    if "nc" not in _CACHE:
        _CACHE["nc"] = _build()
    nc = _CACHE["nc"]

    xt_in = np.ascontiguousarray(x_in.T)
    xt_tg = np.ascontiguousarray(x_tg.T)
    ms_in = (-0.5 * sq_in)[None, :].astype(np.float32)
    ms_tg = (-0.5 * sq_tg)[None, :].astype(np.float32)
    ones = np.ones((1, 128), np.float32)

    in_maps = []
    for c in range(NCORES):
        qi = np.zeros((D, RPAD), np.float32)
        qt = np.zeros((D, RPAD), np.float32)
        qi[:, :RPC] = xt_in[:, c * RPC : (c + 1) * RPC]
        qt[:, :RPC] = xt_tg[:, c * RPC : (c + 1) * RPC]
        in_maps.append(
            {
                "xt_in": xt_in, "xt_tg": xt_tg,
                "q_in": qi, "q_tg": qt,
                "ms_in": ms_in, "ms_tg": ms_tg, "ones": ones,
            }
        )

    import time

    t0 = time.time()
    res = run_bass_kernel_spmd(nc, in_maps, core_ids=list(range(NCORES)))
    _CACHE["wall_s"] = time.time() - t0
    _CACHE["exec_time_ns"] = res.exec_time_ns

    total = 0.0
    n_flag = 0
    for c in range(NCORES):
        o = res.results[c]["out"][:RPC]  # [1250, 8]
        acc, c15A, c16A, zA, c15B, c16B, zB = (o[:, i] for i in range(7))
        tA = 0.5 * (c15A + c16A)
        tB = 0.5 * (c15B + c16B)
        flag = (zA >= tA) | (zB >= tB) | (c15A == c16A) | (c15B == c16B)
        ov = (acc + KNN) * 0.5
        for i in np.nonzero(flag)[0]:
            r = c * RPC + int(i)
            ov[i] = _host_row_overlap(x_in, x_tg, sq_in, sq_tg, r, k)
            n_flag += 1
        total += float(ov.sum())
    _CACHE["n_flag"] = n_flag
    return np.float32(1.0 - total / np.float32(N * k))



# revision 2
# speedup vs baseline: 9.3318x; 9.3318x over previous
"""KNN overlap loss on 8 Trainium2 NeuronCores.

loss = 1 - |top15(input) ∩ top15(target)| / (N*k), per-row index-set overlap.

v2 strategy (vs baseline which shipped full replicated matrices to every
core, ~93MB through the axon tunnel): ship only per-core SHARDS in bf16 and
AllGather on device (~5.3MB upload).

Per-core input pk [260, 1280] bf16 (shard c = rows c*1250..(c+1)*1250):
  rows   0..127 : x_in shard transposed  [D=128, 1250] (cols 1250..1280 zero)
  rows 128..255 : x_tg shard transposed
  row  256/257  : hi/lo bf16 split of -0.5||x_in_j||^2 (pad cols = -1e30)
  row  258/259  : hi/lo bf16 split of -0.5||x_tg_j||^2 (pad cols = -1e30)
Device: AllGather pk over 8 cores -> [2080, 1280]; unpack to SBUF
  xt_full [128, 10240] per matrix (j-axis = 8 chunks of 1280, 240 dead
  columns whose ms = -1e30 keeps them out of every top-k and count).

Per 128-row query block (10 blocks of own padded 1280 rows), per matrix:
  e[q, j] = x_q · x_j - 0.5||x_j||^2  (row-constant term dropped; bf16
  matmul, K=2 ones-matmul accumulates the hi/lo norm rows into PSUM).
  Selection without indices: per 512-wide segment top-8 (DVE max8) -> 160
  candidates; c15/c16 = 15th/16th largest (max8 + match_replace + max8);
  threshold t' = (c15+c16)/2; then
    overlap_row = sum_j [e_in >= t'_in] * sign(e_tg - t'_tg) = 2*ov - 15.
  Exactness guard: z = max over segments of the segment's 8th-largest.
  If z >= t' or c15 == c16 the candidate set may have missed a top-15
  member -> row flagged, host recomputes that row exactly (rare).

bf16 input rounding perturbs top-15 sets of ~6% of rows but the overlap
count is statistically unchanged (measured: rel err 1.4e-5 on the loss,
tolerance 2e-2).
"""

import sys

sys.path.insert(0, "/opt/trn_rl_repo")

import numpy as np
import ml_dtypes

BF16 = ml_dtypes.bfloat16

N = 10000
D = 128
KNN = 15
NCORES = 8
RPC = N // NCORES          # rows per core = 1250
SPAD = 1280                # shard padded to 10 blocks of 128
NBLK = SPAD // 128         # 10
PKROWS = 2 * D + 4         # 260 packed rows per core
NP = NCORES * SPAD         # 10240 j-columns after gather
TW = 512                   # tile width (exactly one PSUM bank of f32)
NT = NP // TW              # 20 tiles

_CACHE = {}


def _build():
    import concourse.bacc as bacc
    import concourse.mybir as mybir
    import concourse.tile as tile

    f32 = mybir.dt.float32
    bf16 = mybir.dt.bfloat16

    nc = bacc.Bacc(None, target_bir_lowering=False, num_devices=NCORES)

    pk = nc.dram_tensor("pk", [PKROWS, SPAD], bf16, kind="ExternalInput")
    ones2 = nc.dram_tensor("ones2", [2, 128], bf16, kind="ExternalInput")
    out_d = nc.dram_tensor("out", [SPAD, 8], f32, kind="ExternalOutput")

    with tile.TileContext(nc) as tc:
        with (
            tc.tile_pool(name="big", bufs=1) as big,
            tc.tile_pool(name="sm", bufs=2) as sm,
            tc.tile_pool(name="dram", bufs=1, space="DRAM") as dram,
            tc.tile_pool(name="ps", bufs=4, space="PSUM") as ps,
        ):
            # ---- gather full matrices from shards ----
            cc_in = dram.tile([PKROWS, SPAD], bf16)
            gath = dram.tile([NCORES * PKROWS, SPAD], bf16, addr_space="Shared")
            nc.gpsimd.dma_start(cc_in[:], pk[:])
            nc.gpsimd.collective_compute(
                "AllGather",
                mybir.AluOpType.bypass,
                replica_groups=[list(range(NCORES))],
                ins=[cc_in[:].opt()],
                outs=[gath[:].opt()],
            )

            xt_in_t = big.tile([128, NP], bf16)
            xt_tg_t = big.tile([128, NP], bf16)
            ms_in_t = big.tile([2, NP], bf16)
            ms_tg_t = big.tile([2, NP], bf16)
            q_in_t = big.tile([128, SPAD], bf16)
            q_tg_t = big.tile([128, SPAD], bf16)
            ones2_t = big.tile([2, 128], bf16)
            e_in_t = big.tile([128, NP], f32)
            e_tg_t = big.tile([128, NP], f32)

            nc.sync.dma_start(q_in_t[:], pk[0:128, :])
            nc.sync.dma_start(q_tg_t[:], pk[128:256, :])
            nc.sync.dma_start(ones2_t[:], ones2[:])
            for c in range(NCORES):
                r0 = c * PKROWS
                cs = slice(c * SPAD, (c + 1) * SPAD)
                nc.sync.dma_start(xt_in_t[:, cs], gath[r0 : r0 + 128, :])
                nc.sync.dma_start(xt_tg_t[:, cs], gath[r0 + 128 : r0 + 256, :])
                nc.sync.dma_start(ms_in_t[:, cs], gath[r0 + 256 : r0 + 258, :])
                nc.sync.dma_start(ms_tg_t[:, cs], gath[r0 + 258 : r0 + 260, :])

            for b in range(NBLK):
                rs = slice(b * 128, (b + 1) * 128)
                # phase A per matrix: matmul tiles -> PSUM -> SBUF + max8 cands
                stats = {}
                for (qt, xtt, mst, et, tagp) in (
                    (q_in_t, xt_in_t, ms_in_t, e_in_t, "pin"),
                    (q_tg_t, xt_tg_t, ms_tg_t, e_tg_t, "ptg"),
                ):
                    cands = sm.tile([128, NT * 8], f32, tag="cands" + tagp)
                    for t in range(NT):
                        cs = slice(t * TW, (t + 1) * TW)
                        pt = ps.tile([128, TW], f32, tag="ps")
                        nc.tensor.matmul(
                            pt[:], qt[:, rs], xtt[:, cs], start=True, stop=False
                        )
                        nc.tensor.matmul(
                            pt[:], ones2_t[:], mst[:, cs], start=False, stop=True
                        )
                        nc.scalar.copy(et[:, cs], pt[:])
                        nc.vector.max(cands[:, t * 8 : (t + 1) * 8], et[:, cs])
                    # threshold from candidates
                    m1 = sm.tile([128, 8], f32, tag="m1" + tagp)
                    mr = sm.tile([128, NT * 8], f32, tag="mr" + tagp)
                    m2 = sm.tile([128, 8], f32, tag="m2" + tagp)
                    zt = sm.tile([128, 8], f32, tag="zt" + tagp)
                    thr = sm.tile([128, 1], f32, tag="thr" + tagp)
                    nthr = sm.tile([128, 1], f32, tag="nthr" + tagp)
                    pre = sm.tile([128, 1], f32, tag="pre" + tagp)
                    nc.vector.max(m1[:], cands[:])
                    nc.vector.match_replace(mr[:], m1[:], cands[:], -1e38)
                    nc.vector.max(m2[:], mr[:])
                    c3 = cands[:].rearrange("p (s e) -> p s e", e=8)
                    nc.vector.max(zt[:], c3[:, :, 7:8])
                    nc.vector.tensor_tensor(
                        pre[:], m2[:, 6:7], m2[:, 7:8], mybir.AluOpType.add
                    )
                    nc.vector.tensor_scalar_mul(thr[:], pre[:], 0.5)
                    nc.vector.tensor_scalar_mul(nthr[:], pre[:], -0.5)
                    stats[tagp] = (thr, nthr, m2, zt)

                thrA, _, m2A, ztA = stats["pin"]
                thrB, nthrB, m2B, ztB = stats["ptg"]

                # phase B: acc_row = sum_j (e_in >= t'A) * sign(e_tg - t'B)
                slots = sm.tile([128, NT], f32, tag="slots")
                for t in range(NT):
                    cs = slice(t * TW, (t + 1) * TW)
                    sg = sm.tile([128, TW], f32, tag="sg")
                    jk = sm.tile([128, TW], f32, tag="jk")
                    nc.scalar.activation(
                        sg[:],
                        e_tg_t[:, cs],
                        mybir.ActivationFunctionType.Sign,
                        bias=nthrB[:],
                        scale=1.0,
                    )
                    nc.vector.scalar_tensor_tensor(
                        jk[:],
                        e_in_t[:, cs],
                        thrA[:],
                        sg[:],
                        mybir.AluOpType.is_ge,
                        mybir.AluOpType.mult,
                        accum_out=slots[:, t : t + 1],
                    )
                ob = sm.tile([128, 8], f32, tag="ob")
                nc.vector.reduce_sum(
                    ob[:, 0:1], slots[:], axis=mybir.AxisListType.X
                )
                nc.vector.tensor_copy(ob[:, 1:2], m2A[:, 6:7])
                nc.vector.tensor_copy(ob[:, 2:3], m2A[:, 7:8])
                nc.vector.tensor_copy(ob[:, 3:4], ztA[:, 0:1])
                nc.vector.tensor_copy(ob[:, 4:5], m2B[:, 6:7])
                nc.vector.tensor_copy(ob[:, 5:6], m2B[:, 7:8])
                nc.vector.tensor_copy(ob[:, 6:7], ztB[:, 0:1])
                nc.vector.tensor_copy(ob[:, 7:8], ztB[:, 0:1])
                nc.sync.dma_start(out_d[rs, :], ob[:])

    nc.finalize()
    return nc


def _host_row_overlap(x_in, x_tg, sq_in, sq_tg, r, k):
    d_in = sq_in[r] + sq_in - 2.0 * (x_in @ x_in[r])
    d_tg = sq_tg[r] + sq_tg - 2.0 * (x_tg @ x_tg[r])
    a = np.argsort(d_in, kind="stable")[:k]
    bb = np.argsort(d_tg, kind="stable")[:k]
    return len(set(a.tolist()) & set(bb.tolist()))


def _split_hi_lo(v):
    """f32 vector -> (hi, lo) bf16 rows with hi+lo ~= v."""
    hi = v.astype(BF16)
    lo = (v - hi.astype(np.float32)).astype(BF16)
    return hi, lo


def kernel(input, target, k):
    from concourse.bass_utils import run_bass_kernel_spmd

    x_in = np.asarray(input, np.float32)
    x_tg = np.asarray(target, np.float32)
    k = int(k)
    sq_in = np.sum(x_in * x_in, axis=1)
    sq_tg = np.sum(x_tg * x_tg, axis=1)

    if k != KNN or x_in.shape != (N, D):
        total = sum(
            _host_row_overlap(x_in, x_tg, sq_in, sq_tg, r, k)
            for r in range(x_in.shape[0])
        )
        return np.float32(1.0 - total / np.float32(x_in.shape[0] * k))

    if "nc" not in _CACHE:
        _CACHE["nc"] = _build()
    nc = _CACHE["nc"]

    xb_in = x_in.astype(BF16)
    xb_tg = x_tg.astype(BF16)
    # norms of the bf16-rounded data (consistent with the device dot products)
    msq_in = -0.5 * np.sum(xb_in.astype(np.float32) ** 2, axis=1)
    msq_tg = -0.5 * np.sum(xb_tg.astype(np.float32) ** 2, axis=1)

    ones2 = np.ones((2, 128), BF16)
    in_maps = []
    for c in range(NCORES):
        rows = slice(c * RPC, (c + 1) * RPC)
        pkc = np.zeros((PKROWS, SPAD), BF16)
        pkc[0:128, :RPC] = xb_in[rows].T
        pkc[128:256, :RPC] = xb_tg[rows].T
        mi = np.full(SPAD, -1e30, np.float32)
        mt = np.full(SPAD, -1e30, np.float32)
        mi[:RPC] = msq_in[rows]
        mt[:RPC] = msq_tg[rows]
        pkc[256], pkc[257] = _split_hi_lo(mi)
        pkc[258], pkc[259] = _split_hi_lo(mt)
        in_maps.append({"pk": pkc, "ones2": ones2})

    import time

    t0 = time.time()
    res = run_bass_kernel_spmd(nc, in_maps, core_ids=list(range(NCORES)))
    _CACHE["wall_s"] = time.time() - t0
    _CACHE["exec_time_ns"] = res.exec_time_ns

    total = 0.0
    n_flag = 0
    for c in range(NCORES):
        o = res.results[c]["out"][:RPC]  # [1250, 8]
        acc, c15A, c16A, zA, c15B, c16B, zB = (o[:, i] for i in range(7))
        tA = 0.5 * (c15A + c16A)
        tB = 0.5 * (c15B + c16B)
        flag = (zA >= tA) | (zB >= tB) | (c15A == c16A) | (c15B == c16B)
        ov = (acc + KNN) * 0.5
        for i in np.nonzero(flag)[0]:
            r = c * RPC + int(i)
            ov[i] = _host_row_overlap(x_in, x_tg, sq_in, sq_tg, r, k)
            n_flag += 1
        total += float(ov.sum())
    _CACHE["n_flag"] = n_flag
    return np.float32(1.0 - total / np.float32(N * k))


# revision 3
# speedup vs baseline: 14.2815x; 1.5304x over previous
"""KNN overlap loss on 8 Trainium2 NeuronCores.

loss = 1 - |top15(input) ∩ top15(target)| / (N*k), per-row index-set overlap.

v3: per-core SHARD upload (fp8-e3m4 data + bf16 norm rows), device-side
AllGather, bf16/fp8 matmuls, threshold-count selection, device-computed
exactness flags, [1280,2] output. Upload ~2.8MB vs 93MB baseline.

Per-core inputs (shard c = rows c*1250..(c+1)*1250 of each matrix):
  pk  [256, 1280] fp8e3m4: rows 0..127 x_in shard transposed (cols
      1250..1280 zero), rows 128..255 x_tg shard transposed.
  msb [4, 1280] bf16: hi/lo split of -0.5||x_j||^2 for in (rows 0,1) and
      tg (rows 2,3), computed from the fp8-rounded data; pad cols = -1e30.
Device: AllGather pk -> [2048,1280], msb -> [32,1280]; unpack to SBUF
  xt_full [128, 10240] per matrix (j = 8 chunks of 1280; 240 dead columns
  whose ms = -1e30 keeps them out of every top-k and count).

Per 128-row query block (10 blocks of own padded 1280 rows), per matrix:
  e[q, j] = x_q · x_j - 0.5||x_j||^2  (row-constant term dropped; fp8
  matmul + K=2 bf16 ones-matmul accumulating hi/lo norm rows into PSUM).
  Selection without indices: per 512-wide segment top-8 (DVE max8) -> 160
  candidates; c15/c16 = 15th/16th largest (max8 + match_replace + max8);
  threshold t' = (c15+c16)/2; then
    overlap_row = sum_j [e_in >= t'_in] * sign(e_tg - t'_tg) = 2*ov - 15.
  Exactness guard (computed on device into out col 1): z = max over
  segments of the segment's 8th-largest; flag if z >= t' or c15 == c16
  for either matrix -> host recomputes that row exactly (rare).

fp8-e3m4 input rounding perturbs borderline top-15 memberships but the
overlap count is statistically unchanged (measured with the coarser
e4m3: rel err 2.1e-5 on the loss; tolerance 2e-2).
"""

import sys

sys.path.insert(0, "/opt/trn_rl_repo")

import numpy as np
import ml_dtypes

BF16 = ml_dtypes.bfloat16
FP8 = ml_dtypes.float8_e3m4

N = 10000
D = 128
KNN = 15
NCORES = 8
RPC = N // NCORES          # rows per core = 1250
SPAD = 1280                # shard padded to 10 blocks of 128
NBLK = SPAD // 128         # 10
NP = NCORES * SPAD         # 10240 j-columns after gather
TW = 512                   # tile width (exactly one PSUM bank of f32)
NT = NP // TW              # 20 tiles

_CACHE = {}


def _build():
    import concourse.bacc as bacc
    import concourse.mybir as mybir
    import concourse.tile as tile

    f32 = mybir.dt.float32
    bf16 = mybir.dt.bfloat16
    f8 = mybir.dt.float8e3

    nc = bacc.Bacc(None, target_bir_lowering=False, num_devices=NCORES)

    pk = nc.dram_tensor("pk", [256, SPAD], f8, kind="ExternalInput")
    msb = nc.dram_tensor("msb", [4, SPAD], bf16, kind="ExternalInput")
    ones2 = nc.dram_tensor("ones2", [2, 128], bf16, kind="ExternalInput")
    out_d = nc.dram_tensor("out", [SPAD, 2], f32, kind="ExternalOutput")

    with tile.TileContext(nc) as tc:
        with (
            tc.tile_pool(name="big", bufs=1) as big,
            tc.tile_pool(name="sm", bufs=2) as sm,
            tc.tile_pool(name="dram", bufs=1, space="DRAM") as dram,
            tc.tile_pool(name="ps", bufs=4, space="PSUM") as ps,
        ):
            # ---- gather full matrices from shards ----
            cc_in = dram.tile([256, SPAD], f8)
            gath = dram.tile([NCORES * 256, SPAD], f8, addr_space="Shared")
            cc_ms = dram.tile([4, SPAD], bf16)
            gathms = dram.tile([NCORES * 4, SPAD], bf16, addr_space="Shared")
            nc.gpsimd.dma_start(cc_in[:], pk[:])
            nc.gpsimd.dma_start(cc_ms[:], msb[:])
            nc.gpsimd.collective_compute(
                "AllGather",
                mybir.AluOpType.bypass,
                replica_groups=[list(range(NCORES))],
                ins=[cc_in[:].opt()],
                outs=[gath[:].opt()],
            )
            nc.gpsimd.collective_compute(
                "AllGather",
                mybir.AluOpType.bypass,
                replica_groups=[list(range(NCORES))],
                ins=[cc_ms[:].opt()],
                outs=[gathms[:].opt()],
            )

            xt_in_t = big.tile([128, NP], f8)
            xt_tg_t = big.tile([128, NP], f8)
            ms_in_t = big.tile([2, NP], bf16)
            ms_tg_t = big.tile([2, NP], bf16)
            q_in_t = big.tile([128, SPAD], f8)
            q_tg_t = big.tile([128, SPAD], f8)
            ones2_t = big.tile([2, 128], bf16)
            e_in_t = big.tile([128, NP], f32)
            e_tg_t = big.tile([128, NP], f32)

            nc.sync.dma_start(q_in_t[:], pk[0:128, :])
            nc.sync.dma_start(q_tg_t[:], pk[128:256, :])
            nc.sync.dma_start(ones2_t[:], ones2[:])
            for c in range(NCORES):
                r0 = c * 256
                m0 = c * 4
                cs = slice(c * SPAD, (c + 1) * SPAD)
                nc.sync.dma_start(xt_in_t[:, cs], gath[r0 : r0 + 128, :])
                nc.sync.dma_start(xt_tg_t[:, cs], gath[r0 + 128 : r0 + 256, :])
                nc.sync.dma_start(ms_in_t[:, cs], gathms[m0 : m0 + 2, :])
                nc.sync.dma_start(ms_tg_t[:, cs], gathms[m0 + 2 : m0 + 4, :])

            for b in range(NBLK):
                rs = slice(b * 128, (b + 1) * 128)
                # phase A per matrix: matmul tiles -> PSUM -> SBUF + max8 cands
                stats = {}
                for (qt, xtt, mst, et, tagp) in (
                    (q_in_t, xt_in_t, ms_in_t, e_in_t, "pin"),
                    (q_tg_t, xt_tg_t, ms_tg_t, e_tg_t, "ptg"),
                ):
                    cands = sm.tile([128, NT * 8], f32, tag="cands" + tagp)
                    for t in range(NT):
                        cs = slice(t * TW, (t + 1) * TW)
                        pt = ps.tile([128, TW], f32, tag="ps")
                        nc.tensor.matmul(
                            pt[:], qt[:, rs], xtt[:, cs], start=True, stop=False
                        )
                        nc.tensor.matmul(
                            pt[:], ones2_t[:], mst[:, cs], start=False, stop=True
                        )
                        nc.scalar.copy(et[:, cs], pt[:])
                        nc.vector.max(cands[:, t * 8 : (t + 1) * 8], et[:, cs])
                    # threshold from candidates
                    m1 = sm.tile([128, 8], f32, tag="m1" + tagp)
                    mr = sm.tile([128, NT * 8], f32, tag="mr" + tagp)
                    m2 = sm.tile([128, 8], f32, tag="m2" + tagp)
                    zt = sm.tile([128, 8], f32, tag="zt" + tagp)
                    thr = sm.tile([128, 1], f32, tag="thr" + tagp)
                    nthr = sm.tile([128, 1], f32, tag="nthr" + tagp)
                    pre = sm.tile([128, 1], f32, tag="pre" + tagp)
                    nc.vector.max(m1[:], cands[:])
                    nc.vector.match_replace(mr[:], m1[:], cands[:], -1e38)
                    nc.vector.max(m2[:], mr[:])
                    c3 = cands[:].rearrange("p (s e) -> p s e", e=8)
                    nc.vector.max(zt[:], c3[:, :, 7:8])
                    nc.vector.tensor_tensor(
                        pre[:], m2[:, 6:7], m2[:, 7:8], mybir.AluOpType.add
                    )
                    nc.vector.tensor_scalar_mul(thr[:], pre[:], 0.5)
                    nc.vector.tensor_scalar_mul(nthr[:], pre[:], -0.5)
                    stats[tagp] = (thr, nthr, m2, zt)

                thrA, _, m2A, ztA = stats["pin"]
                thrB, nthrB, m2B, ztB = stats["ptg"]

                # phase B: acc_row = sum_j (e_in >= t'A) * sign(e_tg - t'B)
                slots = sm.tile([128, NT], f32, tag="slots")
                for t in range(NT):
                    cs = slice(t * TW, (t + 1) * TW)
                    sg = sm.tile([128, TW], f32, tag="sg")
                    jk = sm.tile([128, TW], f32, tag="jk")
                    nc.scalar.activation(
                        sg[:],
                        e_tg_t[:, cs],
                        mybir.ActivationFunctionType.Sign,
                        bias=nthrB[:],
                        scale=1.0,
                    )
                    nc.vector.scalar_tensor_tensor(
                        jk[:],
                        e_in_t[:, cs],
                        thrA[:],
                        sg[:],
                        mybir.AluOpType.is_ge,
                        mybir.AluOpType.mult,
                        accum_out=slots[:, t : t + 1],
                    )
                # out col 0: acc; col 1: exactness flag (>0 -> host recompute)
                ob = sm.tile([128, 2], f32, tag="ob")
                f1 = sm.tile([128, 1], f32, tag="f1")
                f2 = sm.tile([128, 1], f32, tag="f2")
                f3 = sm.tile([128, 1], f32, tag="f3")
                f4 = sm.tile([128, 1], f32, tag="f4")
                nc.vector.reduce_sum(
                    ob[:, 0:1], slots[:], axis=mybir.AxisListType.X
                )
                nc.vector.tensor_tensor(
                    f1[:], ztA[:, 0:1], thrA[:], mybir.AluOpType.is_ge
                )
                nc.vector.tensor_tensor(
                    f2[:], ztB[:, 0:1], thrB[:], mybir.AluOpType.is_ge
                )
                nc.vector.tensor_tensor(
                    f3[:], m2A[:, 6:7], m2A[:, 7:8], mybir.AluOpType.is_equal
                )
                nc.vector.tensor_tensor(
                    f4[:], m2B[:, 6:7], m2B[:, 7:8], mybir.AluOpType.is_equal
                )
                nc.vector.tensor_tensor(f1[:], f1[:], f2[:], mybir.AluOpType.add)
                nc.vector.tensor_tensor(f3[:], f3[:], f4[:], mybir.AluOpType.add)
                nc.vector.tensor_tensor(
                    ob[:, 1:2], f1[:], f3[:], mybir.AluOpType.add
                )
                nc.sync.dma_start(out_d[rs, :], ob[:])

    nc.finalize()
    return nc


def _host_row_overlap(x_in, x_tg, sq_in, sq_tg, r, k):
    d_in = sq_in[r] + sq_in - 2.0 * (x_in @ x_in[r])
    d_tg = sq_tg[r] + sq_tg - 2.0 * (x_tg @ x_tg[r])
    a = np.argsort(d_in, kind="stable")[:k]
    bb = np.argsort(d_tg, kind="stable")[:k]
    return len(set(a.tolist()) & set(bb.tolist()))


def _split_hi_lo(v):
    """f32 vector -> (hi, lo) bf16 rows with hi+lo ~= v."""
    hi = v.astype(BF16)
    lo = (v - hi.astype(np.float32)).astype(BF16)
    return hi, lo


def kernel(input, target, k):
    from concourse.bass_utils import run_bass_kernel_spmd

    x_in = np.asarray(input, np.float32)
    x_tg = np.asarray(target, np.float32)
    k = int(k)
    sq_in = np.sum(x_in * x_in, axis=1)
    sq_tg = np.sum(x_tg * x_tg, axis=1)

    if k != KNN or x_in.shape != (N, D):
        total = sum(
            _host_row_overlap(x_in, x_tg, sq_in, sq_tg, r, k)
            for r in range(x_in.shape[0])
        )
        return np.float32(1.0 - total / np.float32(x_in.shape[0] * k))

    if "nc" not in _CACHE:
        _CACHE["nc"] = _build()
    nc = _CACHE["nc"]

    x8_in = x_in.astype(FP8)
    x8_tg = x_tg.astype(FP8)
    # norms of the fp8-rounded data (consistent with the device dot products)
    msq_in = -0.5 * np.sum(x8_in.astype(np.float32) ** 2, axis=1)
    msq_tg = -0.5 * np.sum(x8_tg.astype(np.float32) ** 2, axis=1)

    ones2 = np.ones((2, 128), BF16)
    in_maps = []
    for c in range(NCORES):
        rows = slice(c * RPC, (c + 1) * RPC)
        pkc = np.zeros((256, SPAD), FP8)
        pkc[0:128, :RPC] = x8_in[rows].T
        pkc[128:256, :RPC] = x8_tg[rows].T
        mi = np.full(SPAD, -1e30, np.float32)
        mt = np.full(SPAD, -1e30, np.float32)
        mi[:RPC] = msq_in[rows]
        mt[:RPC] = msq_tg[rows]
        msbc = np.zeros((4, SPAD), BF16)
        msbc[0], msbc[1] = _split_hi_lo(mi)
        msbc[2], msbc[3] = _split_hi_lo(mt)
        in_maps.append({"pk": pkc, "msb": msbc, "ones2": ones2})

    import time

    t0 = time.time()
    res = run_bass_kernel_spmd(nc, in_maps, core_ids=list(range(NCORES)))
    _CACHE["wall_s"] = time.time() - t0
    _CACHE["exec_time_ns"] = res.exec_time_ns

    total = 0.0
    n_flag = 0
    for c in range(NCORES):
        o = res.results[c]["out"][:RPC]  # [1250, 2]
        ov = (o[:, 0] + KNN) * 0.5
        for i in np.nonzero(o[:, 1] > 0)[0]:
            r = c * RPC + int(i)
            ov[i] = _host_row_overlap(x_in, x_tg, sq_in, sq_tg, r, k)
            n_flag += 1
        total += float(ov.sum())
    _CACHE["n_flag"] = n_flag
    return np.float32(1.0 - total / np.float32(N * k))


# revision 4
# speedup vs baseline: 26.7521x; 1.8732x over previous
"""KNN overlap loss on 8 Trainium2 NeuronCores.

loss = 1 - |top15(input) ∩ top15(target)| / (N*k), per-row index-set overlap.

v3: per-core SHARD upload (fp8-e3m4 data + bf16 norm rows), device-side
AllGather, bf16/fp8 matmuls, threshold-count selection, device-computed
exactness flags, [1280,2] output. Upload ~2.8MB vs 93MB baseline.

Per-core inputs (shard c = rows c*1250..(c+1)*1250 of each matrix):
  pk  [256, 1280] fp8e3m4: rows 0..127 x_in shard transposed (cols
      1250..1280 zero), rows 128..255 x_tg shard transposed.
  msb [4, 1280] bf16: hi/lo split of -0.5||x_j||^2 for in (rows 0,1) and
      tg (rows 2,3), computed from the fp8-rounded data; pad cols = -1e30.
Device: AllGather pk -> [2048,1280], msb -> [32,1280]; unpack to SBUF
  xt_full [128, 10240] per matrix (j = 8 chunks of 1280; 240 dead columns
  whose ms = -1e30 keeps them out of every top-k and count).

Per 128-row query block (10 blocks of own padded 1280 rows), per matrix:
  e[q, j] = x_q · x_j - 0.5||x_j||^2  (row-constant term dropped; fp8
  matmul + K=2 bf16 ones-matmul accumulating hi/lo norm rows into PSUM).
  Selection without indices: per 512-wide segment top-8 (DVE max8) -> 160
  candidates; c15/c16 = 15th/16th largest (max8 + match_replace + max8);
  threshold t' = (c15+c16)/2; then
    overlap_row = sum_j [e_in >= t'_in] * sign(e_tg - t'_tg) = 2*ov - 15.
  Exactness guard (computed on device into out col 1): z = max over
  segments of the segment's 8th-largest; flag if z >= t' or c15 == c16
  for either matrix -> host recomputes that row exactly (rare).

fp8-e3m4 input rounding perturbs borderline top-15 memberships but the
overlap count is statistically unchanged (measured with the coarser
e4m3: rel err 2.1e-5 on the loss; tolerance 2e-2).
"""

import sys

sys.path.insert(0, "/opt/trn_rl_repo")

import numpy as np
import ml_dtypes

try:
    import jax

    jax.config.update("jax_compilation_cache_dir", "/tmp/jax_cc_cache")
    jax.config.update("jax_persistent_cache_min_entry_size_bytes", 0)
    jax.config.update("jax_persistent_cache_min_compile_time_secs", 0.0)
except Exception:
    pass

BF16 = ml_dtypes.bfloat16
FP8 = ml_dtypes.float8_e3m4

N = 10000
D = 128
KNN = 15
NCORES = 8
RPC = N // NCORES          # rows per core = 1250
SPAD = 1280                # shard padded to 10 blocks of 128
NBLK = SPAD // 128         # 10
NP = NCORES * SPAD         # 10240 j-columns after gather
TW = 512                   # tile width (exactly one PSUM bank of f32)
NT = NP // TW              # 20 tiles

_CACHE = {}


def _build():
    import concourse.bacc as bacc
    import concourse.mybir as mybir
    import concourse.tile as tile

    f32 = mybir.dt.float32
    bf16 = mybir.dt.bfloat16
    f8 = mybir.dt.float8e3

    nc = bacc.Bacc(None, target_bir_lowering=False, num_devices=NCORES)

    pk = nc.dram_tensor("pk", [256, SPAD], f8, kind="ExternalInput")
    msb = nc.dram_tensor("msb", [4, SPAD], bf16, kind="ExternalInput")
    ones2 = nc.dram_tensor("ones2", [2, 128], bf16, kind="ExternalInput")
    out_d = nc.dram_tensor("out", [SPAD, 2], f32, kind="ExternalOutput")

    with tile.TileContext(nc) as tc:
        with (
            tc.tile_pool(name="big", bufs=1) as big,
            tc.tile_pool(name="sm", bufs=2) as sm,
            tc.tile_pool(name="dram", bufs=1, space="DRAM") as dram,
            tc.tile_pool(name="ps", bufs=4, space="PSUM") as ps,
        ):
            # ---- gather full matrices from shards ----
            cc_in = dram.tile([256, SPAD], f8)
            gath = dram.tile([NCORES * 256, SPAD], f8, addr_space="Shared")
            cc_ms = dram.tile([4, SPAD], bf16)
            gathms = dram.tile([NCORES * 4, SPAD], bf16, addr_space="Shared")
            nc.gpsimd.dma_start(cc_in[:], pk[:])
            nc.gpsimd.dma_start(cc_ms[:], msb[:])
            nc.gpsimd.collective_compute(
                "AllGather",
                mybir.AluOpType.bypass,
                replica_groups=[list(range(NCORES))],
                ins=[cc_in[:].opt()],
                outs=[gath[:].opt()],
            )
            nc.gpsimd.collective_compute(
                "AllGather",
                mybir.AluOpType.bypass,
                replica_groups=[list(range(NCORES))],
                ins=[cc_ms[:].opt()],
                outs=[gathms[:].opt()],
            )

            xt_in_t = big.tile([128, NP], f8)
            xt_tg_t = big.tile([128, NP], f8)
            ms_in_t = big.tile([2, NP], bf16)
            ms_tg_t = big.tile([2, NP], bf16)
            q_in_t = big.tile([128, SPAD], f8)
            q_tg_t = big.tile([128, SPAD], f8)
            ones2_t = big.tile([2, 128], bf16)
            e_in_t = big.tile([128, NP], f32)
            e_tg_t = big.tile([128, NP], f32)

            nc.sync.dma_start(q_in_t[:], pk[0:128, :])
            nc.sync.dma_start(q_tg_t[:], pk[128:256, :])
            nc.sync.dma_start(ones2_t[:], ones2[:])
            for c in range(NCORES):
                r0 = c * 256
                m0 = c * 4
                cs = slice(c * SPAD, (c + 1) * SPAD)
                nc.sync.dma_start(xt_in_t[:, cs], gath[r0 : r0 + 128, :])
                nc.sync.dma_start(xt_tg_t[:, cs], gath[r0 + 128 : r0 + 256, :])
                nc.sync.dma_start(ms_in_t[:, cs], gathms[m0 : m0 + 2, :])
                nc.sync.dma_start(ms_tg_t[:, cs], gathms[m0 + 2 : m0 + 4, :])

            for b in range(NBLK):
                rs = slice(b * 128, (b + 1) * 128)
                # phase A per matrix: matmul tiles -> PSUM -> SBUF + max8 cands
                stats = {}
                for (qt, xtt, mst, et, tagp) in (
                    (q_in_t, xt_in_t, ms_in_t, e_in_t, "pin"),
                    (q_tg_t, xt_tg_t, ms_tg_t, e_tg_t, "ptg"),
                ):
                    cands = sm.tile([128, NT * 8], f32, tag="cands" + tagp)
                    for t in range(NT):
                        cs = slice(t * TW, (t + 1) * TW)
                        pt = ps.tile([128, TW], f32, tag="ps")
                        nc.tensor.matmul(
                            pt[:], qt[:, rs], xtt[:, cs], start=True, stop=False
                        )
                        nc.tensor.matmul(
                            pt[:], ones2_t[:], mst[:, cs], start=False, stop=True
                        )
                        nc.scalar.copy(et[:, cs], pt[:])
                        nc.vector.max(cands[:, t * 8 : (t + 1) * 8], et[:, cs])
                    # threshold from candidates
                    m1 = sm.tile([128, 8], f32, tag="m1" + tagp)
                    mr = sm.tile([128, NT * 8], f32, tag="mr" + tagp)
                    m2 = sm.tile([128, 8], f32, tag="m2" + tagp)
                    zt = sm.tile([128, 8], f32, tag="zt" + tagp)
                    thr = sm.tile([128, 1], f32, tag="thr" + tagp)
                    nthr = sm.tile([128, 1], f32, tag="nthr" + tagp)
                    pre = sm.tile([128, 1], f32, tag="pre" + tagp)
                    nc.vector.max(m1[:], cands[:])
                    nc.vector.match_replace(mr[:], m1[:], cands[:], -1e38)
                    nc.vector.max(m2[:], mr[:])
                    c3 = cands[:].rearrange("p (s e) -> p s e", e=8)
                    nc.vector.max(zt[:], c3[:, :, 7:8])
                    nc.vector.tensor_tensor(
                        pre[:], m2[:, 6:7], m2[:, 7:8], mybir.AluOpType.add
                    )
                    nc.vector.tensor_scalar_mul(thr[:], pre[:], 0.5)
                    nc.vector.tensor_scalar_mul(nthr[:], pre[:], -0.5)
                    stats[tagp] = (thr, nthr, m2, zt)

                thrA, _, m2A, ztA = stats["pin"]
                thrB, nthrB, m2B, ztB = stats["ptg"]

                # phase B: acc_row = sum_j (e_in >= t'A) * sign(e_tg - t'B)
                slots = sm.tile([128, NT], f32, tag="slots")
                for t in range(NT):
                    cs = slice(t * TW, (t + 1) * TW)
                    sg = sm.tile([128, TW], f32, tag="sg")
                    jk = sm.tile([128, TW], f32, tag="jk")
                    nc.scalar.activation(
                        sg[:],
                        e_tg_t[:, cs],
                        mybir.ActivationFunctionType.Sign,
                        bias=nthrB[:],
                        scale=1.0,
                    )
                    nc.vector.scalar_tensor_tensor(
                        jk[:],
                        e_in_t[:, cs],
                        thrA[:],
                        sg[:],
                        mybir.AluOpType.is_ge,
                        mybir.AluOpType.mult,
                        accum_out=slots[:, t : t + 1],
                    )
                # out col 0: acc; col 1: exactness flag (>0 -> host recompute)
                ob = sm.tile([128, 2], f32, tag="ob")
                f1 = sm.tile([128, 1], f32, tag="f1")
                f2 = sm.tile([128, 1], f32, tag="f2")
                f3 = sm.tile([128, 1], f32, tag="f3")
                f4 = sm.tile([128, 1], f32, tag="f4")
                nc.vector.reduce_sum(
                    ob[:, 0:1], slots[:], axis=mybir.AxisListType.X
                )
                nc.vector.tensor_tensor(
                    f1[:], ztA[:, 0:1], thrA[:], mybir.AluOpType.is_ge
                )
                nc.vector.tensor_tensor(
                    f2[:], ztB[:, 0:1], thrB[:], mybir.AluOpType.is_ge
                )
                nc.vector.tensor_tensor(
                    f3[:], m2A[:, 6:7], m2A[:, 7:8], mybir.AluOpType.is_equal
                )
                nc.vector.tensor_tensor(
                    f4[:], m2B[:, 6:7], m2B[:, 7:8], mybir.AluOpType.is_equal
                )
                nc.vector.tensor_tensor(f1[:], f1[:], f2[:], mybir.AluOpType.add)
                nc.vector.tensor_tensor(f3[:], f3[:], f4[:], mybir.AluOpType.add)
                nc.vector.tensor_tensor(
                    ob[:, 1:2], f1[:], f3[:], mybir.AluOpType.add
                )
                nc.sync.dma_start(out_d[rs, :], ob[:])

    nc.finalize()
    return nc


def _host_row_overlap(x_in, x_tg, sq_in, sq_tg, r, k):
    d_in = sq_in[r] + sq_in - 2.0 * (x_in @ x_in[r])
    d_tg = sq_tg[r] + sq_tg - 2.0 * (x_tg @ x_tg[r])
    a = np.argsort(d_in, kind="stable")[:k]
    bb = np.argsort(d_tg, kind="stable")[:k]
    return len(set(a.tolist()) & set(bb.tolist()))


def _split_hi_lo(v):
    """f32 vector -> (hi, lo) bf16 rows with hi+lo ~= v."""
    hi = v.astype(BF16)
    lo = (v - hi.astype(np.float32)).astype(BF16)
    return hi, lo


def kernel(input, target, k):
    from concourse.bass_utils import run_bass_kernel_spmd

    x_in = np.asarray(input, np.float32)
    x_tg = np.asarray(target, np.float32)
    k = int(k)
    sq_in = np.sum(x_in * x_in, axis=1)
    sq_tg = np.sum(x_tg * x_tg, axis=1)

    if k != KNN or x_in.shape != (N, D):
        total = sum(
            _host_row_overlap(x_in, x_tg, sq_in, sq_tg, r, k)
            for r in range(x_in.shape[0])
        )
        return np.float32(1.0 - total / np.float32(x_in.shape[0] * k))

    if "nc" not in _CACHE:
        _CACHE["nc"] = _build()
    nc = _CACHE["nc"]

    x8_in = x_in.astype(FP8)
    x8_tg = x_tg.astype(FP8)
    # norms of the fp8-rounded data (consistent with the device dot products)
    msq_in = -0.5 * np.sum(x8_in.astype(np.float32) ** 2, axis=1)
    msq_tg = -0.5 * np.sum(x8_tg.astype(np.float32) ** 2, axis=1)

    ones2 = np.ones((2, 128), BF16)
    in_maps = []
    for c in range(NCORES):
        rows = slice(c * RPC, (c + 1) * RPC)
        pkc = np.zeros((256, SPAD), FP8)
        pkc[0:128, :RPC] = x8_in[rows].T
        pkc[128:256, :RPC] = x8_tg[rows].T
        mi = np.full(SPAD, -1e30, np.float32)
        mt = np.full(SPAD, -1e30, np.float32)
        mi[:RPC] = msq_in[rows]
        mt[:RPC] = msq_tg[rows]
        msbc = np.zeros((4, SPAD), BF16)
        msbc[0], msbc[1] = _split_hi_lo(mi)
        msbc[2], msbc[3] = _split_hi_lo(mt)
        in_maps.append({"pk": pkc, "msb": msbc, "ones2": ones2})

    import time

    t0 = time.time()
    res = run_bass_kernel_spmd(nc, in_maps, core_ids=list(range(NCORES)))
    _CACHE["wall_s"] = time.time() - t0
    _CACHE["exec_time_ns"] = res.exec_time_ns

    total = 0.0
    n_flag = 0
    for c in range(NCORES):
        o = res.results[c]["out"][:RPC]  # [1250, 2]
        ov = (o[:, 0] + KNN) * 0.5
        for i in np.nonzero(o[:, 1] > 0)[0]:
            r = c * RPC + int(i)
            ov[i] = _host_row_overlap(x_in, x_tg, sq_in, sq_tg, r, k)
            n_flag += 1
        total += float(ov.sum())
    _CACHE["n_flag"] = n_flag
    return np.float32(1.0 - total / np.float32(N * k))


# revision 5
# speedup vs baseline: 32.2760x; 1.2065x over previous
"""KNN overlap loss on 8 Trainium2 NeuronCores.

loss = 1 - |top15(input) ∩ top15(target)| / (N*k), per-row index-set overlap.

v4: per-core SHARD upload (fp8-e3m4 data + bf16 norm rows), device-side
AllGather, fp8/bf16 matmuls, threshold-count selection, device-computed
exactness flags, [1280,2] output, hardware For_i loop over the 10 query
blocks (small program -> fast build + compile), jax persistent
compilation cache so repeat processes skip the NEFF compile entirely.

Per-core inputs (shard c = rows c*1250..(c+1)*1250 of each matrix):
  pk  [256, 1280] fp8e3m4: rows 0..127 x_in shard transposed (cols
      1250..1280 zero), rows 128..255 x_tg shard transposed.
  msb [4, 1280] bf16: hi/lo split of -0.5||x_j||^2 for in (rows 0,1) and
      tg (rows 2,3), computed from the fp8-rounded data; pad cols = -1e30.
Device: AllGather pk -> [2048,1280], msb -> [32,1280]; unpack to SBUF
  xt_full [128, 10240] per matrix (j = 8 chunks of 1280; 240 dead columns
  whose ms = -1e30 keeps them out of every top-k and count).

Per 128-row query block (For_i over 10 blocks of own padded 1280 rows),
per matrix:
  e[q, j] = x_q · x_j - 0.5||x_j||^2  (row-constant term dropped; fp8
  matmul + K=2 bf16 ones-matmul accumulating hi/lo norm rows into PSUM).
  Selection without indices: per 512-wide segment top-8 (DVE max8) -> 160
  candidates; c15/c16 = 15th/16th largest (max8 + match_replace + max8);
  threshold t' = (c15+c16)/2; then
    overlap_row = sum_j [e_in >= t'_in] * sign(e_tg - t'_tg) = 2*ov - 15.
  Exactness guard (computed on device into out col 1): z = max over
  segments of the segment's 8th-largest; flag if z >= t' or c15 == c16
  for either matrix -> host recomputes that row exactly (rare).

fp8-e3m4 input rounding perturbs borderline top-15 memberships but the
overlap count is statistically unchanged (measured with the coarser
e4m3: rel err 2.1e-5 on the loss; tolerance 2e-2).
"""

import sys

sys.path.insert(0, "/opt/trn_rl_repo")

import numpy as np
import ml_dtypes

try:
    import jax

    jax.config.update("jax_compilation_cache_dir", "/tmp/jax_cc_cache")
    jax.config.update("jax_persistent_cache_min_entry_size_bytes", 0)
    jax.config.update("jax_persistent_cache_min_compile_time_secs", 0.0)
except Exception:
    pass

BF16 = ml_dtypes.bfloat16
FP8 = ml_dtypes.float8_e3m4

N = 10000
D = 128
KNN = 15
NCORES = 8
RPC = N // NCORES          # rows per core = 1250
SPAD = 1280                # shard padded to 10 blocks of 128
NBLK = SPAD // 128         # 10
NP = NCORES * SPAD         # 10240 j-columns after gather
TW = 512                   # tile width (exactly one PSUM bank of f32)
NT = NP // TW              # 20 tiles

_CACHE = {}


def _build():
    import concourse.bacc as bacc
    import concourse.mybir as mybir
    import concourse.tile as tile
    from concourse.bass import ds

    f32 = mybir.dt.float32
    bf16 = mybir.dt.bfloat16
    f8 = mybir.dt.float8e3

    nc = bacc.Bacc(None, target_bir_lowering=False, num_devices=NCORES)

    pk = nc.dram_tensor("pk", [256, SPAD], f8, kind="ExternalInput")
    msb = nc.dram_tensor("msb", [4, SPAD], bf16, kind="ExternalInput")
    ones2 = nc.dram_tensor("ones2", [2, 128], bf16, kind="ExternalInput")
    out_d = nc.dram_tensor("out", [SPAD, 2], f32, kind="ExternalOutput")

    with tile.TileContext(nc) as tc:
        with (
            tc.tile_pool(name="big", bufs=1) as big,
            tc.tile_pool(name="sm", bufs=2) as sm,
            tc.tile_pool(name="dram", bufs=1, space="DRAM") as dram,
            tc.tile_pool(name="ps", bufs=4, space="PSUM") as ps,
        ):
            # ---- gather full matrices from shards ----
            cc_in = dram.tile([256, SPAD], f8)
            gath = dram.tile([NCORES * 256, SPAD], f8, addr_space="Shared")
            cc_ms = dram.tile([4, SPAD], bf16)
            gathms = dram.tile([NCORES * 4, SPAD], bf16, addr_space="Shared")
            nc.gpsimd.dma_start(cc_in[:], pk[:])
            nc.gpsimd.dma_start(cc_ms[:], msb[:])
            nc.gpsimd.collective_compute(
                "AllGather",
                mybir.AluOpType.bypass,
                replica_groups=[list(range(NCORES))],
                ins=[cc_in[:].opt()],
                outs=[gath[:].opt()],
            )
            nc.gpsimd.collective_compute(
                "AllGather",
                mybir.AluOpType.bypass,
                replica_groups=[list(range(NCORES))],
                ins=[cc_ms[:].opt()],
                outs=[gathms[:].opt()],
            )

            xt_in_t = big.tile([128, NP], f8)
            xt_tg_t = big.tile([128, NP], f8)
            ms_in_t = big.tile([2, NP], bf16)
            ms_tg_t = big.tile([2, NP], bf16)
            q_in_t = big.tile([128, SPAD], f8)
            q_tg_t = big.tile([128, SPAD], f8)
            ones2_t = big.tile([2, 128], bf16)
            e_in_t = big.tile([128, NP], f32)
            e_tg_t = big.tile([128, NP], f32)

            nc.sync.dma_start(q_in_t[:], pk[0:128, :])
            nc.sync.dma_start(q_tg_t[:], pk[128:256, :])
            nc.sync.dma_start(ones2_t[:], ones2[:])
            for c in range(NCORES):
                r0 = c * 256
                m0 = c * 4
                cs = slice(c * SPAD, (c + 1) * SPAD)
                nc.sync.dma_start(xt_in_t[:, cs], gath[r0 : r0 + 128, :])
                nc.sync.dma_start(xt_tg_t[:, cs], gath[r0 + 128 : r0 + 256, :])
                nc.sync.dma_start(ms_in_t[:, cs], gathms[m0 : m0 + 2, :])
                nc.sync.dma_start(ms_tg_t[:, cs], gathms[m0 + 2 : m0 + 4, :])

            with tc.For_i(0, NBLK) as b:
                roff = b * 128
                # stage this block's query columns at a fixed SBUF address
                qs_in = sm.tile([128, 128], f8, tag="qsin")
                qs_tg = sm.tile([128, 128], f8, tag="qstg")
                nc.sync.dma_start(qs_in[:], q_in_t[:, ds(roff, 128)])
                nc.sync.dma_start(qs_tg[:], q_tg_t[:, ds(roff, 128)])

                # phase A per matrix: matmul tiles -> PSUM -> SBUF + max8 cands
                stats = {}
                for (qs, xtt, mst, et, tagp) in (
                    (qs_in, xt_in_t, ms_in_t, e_in_t, "pin"),
                    (qs_tg, xt_tg_t, ms_tg_t, e_tg_t, "ptg"),
                ):
                    cands = sm.tile([128, NT * 8], f32, tag="cands" + tagp)
                    for t in range(NT):
                        cs = slice(t * TW, (t + 1) * TW)
                        pt = ps.tile([128, TW], f32, tag="ps")
                        nc.tensor.matmul(
                            pt[:], qs[:], xtt[:, cs], start=True, stop=False
                        )
                        nc.tensor.matmul(
                            pt[:], ones2_t[:], mst[:, cs], start=False, stop=True
                        )
                        nc.scalar.copy(et[:, cs], pt[:])
                        nc.vector.max(cands[:, t * 8 : (t + 1) * 8], et[:, cs])
                    # threshold from candidates
                    m1 = sm.tile([128, 8], f32, tag="m1" + tagp)
                    mr = sm.tile([128, NT * 8], f32, tag="mr" + tagp)
                    m2 = sm.tile([128, 8], f32, tag="m2" + tagp)
                    zt = sm.tile([128, 8], f32, tag="zt" + tagp)
                    thr = sm.tile([128, 1], f32, tag="thr" + tagp)
                    nthr = sm.tile([128, 1], f32, tag="nthr" + tagp)
                    pre = sm.tile([128, 1], f32, tag="pre" + tagp)
                    nc.vector.max(m1[:], cands[:])
                    nc.vector.match_replace(mr[:], m1[:], cands[:], -1e38)
                    nc.vector.max(m2[:], mr[:])
                    c3 = cands[:].rearrange("p (s e) -> p s e", e=8)
                    nc.vector.max(zt[:], c3[:, :, 7:8])
                    nc.vector.tensor_tensor(
                        pre[:], m2[:, 6:7], m2[:, 7:8], mybir.AluOpType.add
                    )
                    nc.vector.tensor_scalar_mul(thr[:], pre[:], 0.5)
                    nc.vector.tensor_scalar_mul(nthr[:], pre[:], -0.5)
                    stats[tagp] = (thr, nthr, m2, zt)

                thrA, _, m2A, ztA = stats["pin"]
                thrB, nthrB, m2B, ztB = stats["ptg"]

                # phase B: acc_row = sum_j (e_in >= t'A) * sign(e_tg - t'B)
                slots = sm.tile([128, NT], f32, tag="slots")
                for t in range(NT):
                    cs = slice(t * TW, (t + 1) * TW)
                    sg = sm.tile([128, TW], f32, tag="sg")
                    jk = sm.tile([128, TW], f32, tag="jk")
                    nc.scalar.activation(
                        sg[:],
                        e_tg_t[:, cs],
                        mybir.ActivationFunctionType.Sign,
                        bias=nthrB[:],
                        scale=1.0,
                    )
                    nc.vector.scalar_tensor_tensor(
                        jk[:],
                        e_in_t[:, cs],
                        thrA[:],
                        sg[:],
                        mybir.AluOpType.is_ge,
                        mybir.AluOpType.mult,
                        accum_out=slots[:, t : t + 1],
                    )
                # out col 0: acc; col 1: exactness flag (>0 -> host recompute)
                ob = sm.tile([128, 2], f32, tag="ob")
                f1 = sm.tile([128, 1], f32, tag="f1")
                f2 = sm.tile([128, 1], f32, tag="f2")
                f3 = sm.tile([128, 1], f32, tag="f3")
                f4 = sm.tile([128, 1], f32, tag="f4")
                nc.vector.reduce_sum(
                    ob[:, 0:1], slots[:], axis=mybir.AxisListType.X
                )
                nc.vector.tensor_tensor(
                    f1[:], ztA[:, 0:1], thrA[:], mybir.AluOpType.is_ge
                )
                nc.vector.tensor_tensor(
                    f2[:], ztB[:, 0:1], thrB[:], mybir.AluOpType.is_ge
                )
                nc.vector.tensor_tensor(
                    f3[:], m2A[:, 6:7], m2A[:, 7:8], mybir.AluOpType.is_equal
                )
                nc.vector.tensor_tensor(
                    f4[:], m2B[:, 6:7], m2B[:, 7:8], mybir.AluOpType.is_equal
                )
                nc.vector.tensor_tensor(f1[:], f1[:], f2[:], mybir.AluOpType.add)
                nc.vector.tensor_tensor(f3[:], f3[:], f4[:], mybir.AluOpType.add)
                nc.vector.tensor_tensor(
                    ob[:, 1:2], f1[:], f3[:], mybir.AluOpType.add
                )
                nc.sync.dma_start(out_d[ds(roff, 128), :], ob[:])

    nc.finalize()
    return nc


def _host_row_overlap(x_in, x_tg, sq_in, sq_tg, r, k):
    d_in = sq_in[r] + sq_in - 2.0 * (x_in @ x_in[r])
    d_tg = sq_tg[r] + sq_tg - 2.0 * (x_tg @ x_tg[r])
    a = np.argsort(d_in, kind="stable")[:k]
    bb = np.argsort(d_tg, kind="stable")[:k]
    return len(set(a.tolist()) & set(bb.tolist()))


def _split_hi_lo(v):
    """f32 vector -> (hi, lo) bf16 rows with hi+lo ~= v."""
    hi = v.astype(BF16)
    lo = (v - hi.astype(np.float32)).astype(BF16)
    return hi, lo


def kernel(input, target, k):
    from concourse.bass_utils import run_bass_kernel_spmd

    x_in = np.asarray(input, np.float32)
    x_tg = np.asarray(target, np.float32)
    k = int(k)
    sq_in = np.sum(x_in * x_in, axis=1)
    sq_tg = np.sum(x_tg * x_tg, axis=1)

    if k != KNN or x_in.shape != (N, D):
        total = sum(
            _host_row_overlap(x_in, x_tg, sq_in, sq_tg, r, k)
            for r in range(x_in.shape[0])
        )
        return np.float32(1.0 - total / np.float32(x_in.shape[0] * k))

    if "nc" not in _CACHE:
        _CACHE["nc"] = _build()
    nc = _CACHE["nc"]

    x8_in = x_in.astype(FP8)
    x8_tg = x_tg.astype(FP8)
    # norms of the fp8-rounded data (consistent with the device dot products)
    msq_in = -0.5 * np.sum(x8_in.astype(np.float32) ** 2, axis=1)
    msq_tg = -0.5 * np.sum(x8_tg.astype(np.float32) ** 2, axis=1)

    ones2 = np.ones((2, 128), BF16)
    in_maps = []
    for c in range(NCORES):
        rows = slice(c * RPC, (c + 1) * RPC)
        pkc = np.zeros((256, SPAD), FP8)
        pkc[0:128, :RPC] = x8_in[rows].T
        pkc[128:256, :RPC] = x8_tg[rows].T
        mi = np.full(SPAD, -1e30, np.float32)
        mt = np.full(SPAD, -1e30, np.float32)
        mi[:RPC] = msq_in[rows]
        mt[:RPC] = msq_tg[rows]
        msbc = np.zeros((4, SPAD), BF16)
        msbc[0], msbc[1] = _split_hi_lo(mi)
        msbc[2], msbc[3] = _split_hi_lo(mt)
        in_maps.append({"pk": pkc, "msb": msbc, "ones2": ones2})

    import time

    t0 = time.time()
    res = run_bass_kernel_spmd(nc, in_maps, core_ids=list(range(NCORES)))
    _CACHE["wall_s"] = time.time() - t0
    _CACHE["exec_time_ns"] = res.exec_time_ns

    total = 0.0
    n_flag = 0
    for c in range(NCORES):
        o = res.results[c]["out"][:RPC]  # [1250, 2]
        ov = (o[:, 0] + KNN) * 0.5
        for i in np.nonzero(o[:, 1] > 0)[0]:
            r = c * RPC + int(i)
            ov[i] = _host_row_overlap(x_in, x_tg, sq_in, sq_tg, r, k)
            n_flag += 1
        total += float(ov.sum())
    _CACHE["n_flag"] = n_flag
    return np.float32(1.0 - total / np.float32(N * k))
